# revision 10
# baseline (speedup 1.0000x reference)
"""Trainium2 Bass kernel for nn_DINA_25503515804209 (sparse_attention).

Math (per batch b, head h):
  M = concat(x1, pad(x2)) in R^{2048 x 64}
  K = (1/8) * M U_h M^T          (2048 x 2048)
  rows_i = max(0, max_{p in allowed(i)} K[i,p])
  cols_p = max(0, max_{i in allowed(p)} K[i,p])
    (leading 848x848 block masked; the reference's mask fill value
     min(relu(K_head0)) is 0 for any real input since relu >= 0 and some
     entry is always <= 0 -- the max(0, .) floor implements it exactly)
  alpha = rows + cols; w1 = softmax(alpha[:1200]); w2 = softmax(alpha[1200:])
  r1 = w1 @ M[:1200]; r2 = w2 @ M[1200:]

Sharding: data-parallel over batch B=8 across the 8 NeuronCores.

Engine split per core (DVE is the bottleneck; everything else feeds it):
  PE   strip matmuls (f32r, both heads) + finalize transposes + tail matmuls
  ACT  PSUM->SBUF fp16 drains of every strip + softmax exp
  DVE  rowmax (2-port ttmax custom op), colmax accumulate (fp16 2x
       tensor_max), finalize reduces, small tail ops
  Pool acc[0:848] seeds and the boundary-strip mask zeroing (memsets)
"""

import json

import numpy as np

B, L1, D1, L2, D2, H, C = 8, 1200, 64, 848, 48, 2, 64
Q = L1 + L2            # 2048
NT = Q // 128          # 16 row tiles
MASKED = L2            # leading 848x848 block is masked
NRESTR = 6             # row tiles 0..5 lie fully inside the masked rows
BND = 6                # row tile 6 straddles the mask boundary (row 848)
BP = MASKED - BND * 128  # = 80: partitions 0:80 of tile 6 are masked rows

_CACHE = {}


# --------------------------------------------------------------------------
# BIR post-processing: this walrus build encodes at most one semaphore wait
# per instruction; Tile emits multi-wait sync_infos.  Hoist excess waits
# into preceding same-engine EventSemaphore instructions (what wait_ge
# emits) -- engine sequencers execute in order, so semantics are identical.
# Also run codegen_inst_isa_subclasses, which populates .instr bytes for
# InstISA subclasses (custom DVE ops); raw Bass does not run that pass and
# walrus fails with "ISA wrong length" on empty instr arrays.
# --------------------------------------------------------------------------
def _split_waits_json(j):
    for fn in j.get("functions", []):
        for blk in fn.get("blocks", []):
            insts = blk.get("instructions")
            if not insts:
                continue
            out = []
            for ins in insts:
                si = ins.get("sync_info")
                waits = (si or {}).get("on_wait") or []
                if len(waits) > 1:
                    for k, wt in enumerate(waits[:-1]):
                        out.append(
                            {
                                "debug": ins.get("debug"),
                                "engine": ins["engine"],
                                "ins": [],
                                "name": f"{ins['name']}_hw{k}",
                                "opcode": "EventSemaphore",
                                "outs": [],
                                "sync_info": {"on_update": [], "on_wait": [wt]},
                            }
                        )
                    si["on_wait"] = waits[-1:]
                ups = (si or {}).get("on_update") or []
                if len(ups) > 1:
                    raise RuntimeError(
                        f"instruction {ins['name']} has {len(ups)} updates"
                    )
                out.append(ins)
            blk["instructions"] = out


def _patch_bass_json(nc):
    import concourse.mybir as mybir

    orig = nc.to_json_bytes
    done = []

    def to_json_bytes_patched():
        if not done:
            mybir.codegen_inst_isa_subclasses(nc)
            done.append(True)
        j = json.loads(orig())
        _split_waits_json(j)
        return json.dumps(j).encode()

    nc.to_json_bytes = to_json_bytes_patched
    return nc


def _ttmax_reduce_op():
    """Fused  out = max(in0, in1);  accum_out = rowmax(out)  custom DVE op.

    Consumes two fp16 streams per cycle (both DVE read ports), so one
    instruction replaces the whole pairwise row-max tree of a strip.
    Registered at runtime through dve_ops' documented extension point
    (the uop program ships in the per-NEFF DVE table)."""
    import numpy as np
    import concourse.dve_ops as dve_ops
    from concourse.dve_spec import Spec, Src0, Src1, maxx, lower
    from concourse.dve_table_gen import dve_ver_for
    from concourse.dve_uop import DveOpSpec

    NAME = "TT_MAX_ROWMAX_ANT"
    if NAME in dve_ops._SUB_OPCODE_FOR_NAME:
        return next(op for op in dve_ops.OPS if op.name == NAME)

    def _ref(in0, in1, c0, c1, c2):
        body = np.maximum(in0.astype(np.float32), in1.astype(np.float32))
        return body, body.reshape(body.shape[0], -1).max(axis=-1, keepdims=True)

    spec = Spec(body=maxx(Src0, Src1), accum=maxx, reference=_ref)
    row = dve_ops._CUSTOM_DVE_ROW_BASE + len(dve_ops.OPS)
    ver = dve_ver_for("TRN2")
    sha = DveOpSpec(
        name=NAME, opcode=row, uops=lower(spec, ver=ver), rd1_en=True
    ).sha(ver)
    op = dve_ops.DveOp(NAME, spec, subdim=False, uops_sha={ver: sha})
    dve_ops.OPS.append(op)
    dve_ops._SUB_OPCODE_FOR_NAME[NAME] = row
    dve_ops.CUSTOM_DVE_SPECS[NAME] = spec
    return op


def _build_nc():
    import concourse.bass as bass
    import concourse.mybir as mybir
    import concourse.tile as tile
    from concourse.dve_ops import TENSOR_MASK_REDUCE
    from concourse.masks import make_identity

    ttmax = _ttmax_reduce_op()

    f32 = mybir.dt.float32
    f32r = mybir.dt.float32r
    f16 = mybir.dt.float16
    AX = mybir.AxisListType
    ALU = mybir.AluOpType
    ACTF = mybir.ActivationFunctionType

    nc = bass.Bass(trn_type="TRN2")

    mt_d = nc.dram_tensor("mt_in", [C, Q], f32, kind="ExternalInput")
    m_d = nc.dram_tensor("m_in", [Q, C], f32, kind="ExternalInput")
    at_d = nc.dram_tensor("at_in", [C, 2, Q], f32, kind="ExternalInput")
    bm_d = nc.dram_tensor("bmask_in", [128, 2], f32, kind="ExternalInput")
    out_d = nc.dram_tensor("out", [C, 4], f32, kind="ExternalOutput")
    spm_d = nc.dram_tensor("spm", [128, 4], f32, kind="ExternalOutput")
    if _CACHE.get("debug"):
        dbg_rows = nc.dram_tensor("dbg_rows", [128, 2, NT], f32, kind="ExternalOutput")
        dbg_cols = nc.dram_tensor("dbg_cols", [128, 2, NT], f32, kind="ExternalOutput")
        dbg_acc = nc.dram_tensor("dbg_acc", [128, 2, Q], f32, kind="ExternalOutput")

    with tile.TileContext(nc) as tc:
        with (
            tc.tile_pool(name="sb", bufs=1) as sb,
            tc.tile_pool(name="dscr", bufs=4) as dscr,
        ):
            # ---- load inputs (f32r tiles loaded directly; PE rounds).
            # A^T = (M U_h)^T is precomputed on the host (33 MFLOP) so the
            # strip matmuls start as soon as the first DMA chunks land.
            # Order: what strip t0 (restricted, cols 848:) needs comes first.
            mtr = sb.tile([C, Q], f32r, tag="mtr")
            atr = sb.tile([C, 2, Q], f32r, tag="atr")
            # strict priority order, single queue: the first strip needs
            # atr[:, :, 0:256] and mtr banks 1-3; everything else after.
            nc.sync.dma_start(
                out=atr[:, :, 0:256], in_=at_d[:, :, 0:256].bitcast(f32r)
            )
            nc.sync.dma_start(out=mtr[:, 512:Q], in_=mt_d[:, 512:Q].bitcast(f32r))
            nc.sync.dma_start(
                out=atr[:, :, 256:1024], in_=at_d[:, :, 256:1024].bitcast(f32r)
            )
            nc.sync.dma_start(
                out=atr[:, :, 1024:Q], in_=at_d[:, :, 1024:Q].bitcast(f32r)
            )
            nc.sync.dma_start(out=mtr[:, 0:512], in_=mt_d[:, 0:512].bitcast(f32r))

            e1200 = sb.tile([128, 1], f32, tag="e1200")
            nc.vector.memset(e1200, float(Q - MASKED))
            bm = sb.tile([128, 2], f32, tag="bm")
            nc.sync.dma_start(out=bm, in_=bm_d[:, :])
            ident16 = sb.tile([128, 128], f16, tag="ident16")
            make_identity(nc, ident16)

            # ---- per-head strip processing -------------------------------
            # rows: 2-port ttmax over the ACT-drained fp16 copy (except the
            # first two h0 strips, which use the 1-port TENSOR_MASK_REDUCE
            # directly on PSUM so the DVE has work before the ACT drain
            # pipeline fills).  cols: fp16 2x tensor_max accumulation into a
            # per-head surface, seeded by strip t0's drain (cols 848:) and a
            # Pool memset (cols 0:848).  Boundary strip t6 drains fully and
            # Pool zeroes its masked block; it is processed LAST in the head
            # so the extra Pool hop never blocks the DVE queue.
            rows0 = sb.tile([128, NT], f32, tag="rows0")
            rows1 = sb.tile([128, NT], f32, tag="rows1")
            cols0 = sb.tile([128, NT], f32, tag="cols0")
            cols1 = sb.tile([128, NT], f32, tag="cols1")
            acc0 = sb.tile([128, Q], f16, tag="acc0")
            acc1 = sb.tile([128, Q], f16, tag="acc1")
            junk = sb.tile([128, Q // 2], f16, tag="junk")
            nc.gpsimd.memset(acc0[:, 0:MASKED], 0.0)
            nc.gpsimd.memset(acc1[:, 0:MASKED], 0.0)

            ORDER = [0, 1, 2, 3, 4, 5, 7, 8, 9, 10, BND, 11, 12, 13, 14, 15]

            alpha_seg = sb.tile([128, 34], f32, tag="alpha_seg")
            s_pm = sb.tile([128, 4], f32, tag="s_pm")
            w34 = sb.tile([128, 34], f32, tag="w34")
            w2 = sb.tile([128, 17, 2], f32, tag="w2")

            with tc.tile_pool(name="psK", bufs=1, space="PSUM") as psK:
                def strip(h, t, k):
                    """k = position in processing order (for psum parity)."""
                    ramp = 4 if h == 0 else 2
                    acc = acc0 if h == 0 else acc1
                    rows = rows0 if h == 0 else rows1
                    isl = slice(128 * t, 128 * (t + 1))
                    lo = MASKED if t < NRESTR else 0
                    mmlo = 512 if t < NRESTR else 0
                    w = Q - lo
                    pkf = psK.tile([128, Q], f32, tag=f"pk{k % 2}",
                                   name=f"pk_{h}_{t}")
                    pk = pkf[:, lo:Q]
                    for j in range(mmlo // 512, 4):
                        nc.tensor.matmul(
                            pkf[:, 512 * j : 512 * (j + 1)],
                            atr[:, h, isl],
                            mtr[:, 512 * j : 512 * (j + 1)],
                            start=True, stop=True,
                        )
                    if k < ramp:
                        # ramp: masked-reduce straight from PSUM (drain +
                        # rowmax in one DVE op, no ACT dependency)
                        if t == 0:
                            dbuf = acc[:, lo:Q]
                        else:
                            dbuf = dscr.tile([128, Q], f16, tag="d",
                                             name=f"d_{h}_{t}")[:, lo:Q]
                        nc.vector._custom_dve(
                            TENSOR_MASK_REDUCE,
                            out=dbuf, in0=pk[:, :], in1=e1200,
                            s0=0.0, s1=0.0, imm2=1.0,
                            accum_out=rows[:, t : t + 1],
                        )
                    else:
                        if t == 0:
                            dbuf = acc[:, lo:Q]
                        else:
                            dbuf = dscr.tile([128, Q], f16, tag="d",
                                             name=f"d_{h}_{t}")[:, lo:Q]
                        nc.scalar.copy(dbuf, pk[:, :])
                        if t == BND:
                            # zero the masked block (rows 768:848 x cols
                            # 0:848); 0 is max-neutral after the relu floor.
                            nc.gpsimd.memset(dbuf[0:64, 0:MASKED], 0.0)
                            nc.gpsimd.memset(dbuf[64:BP, 0:MASKED], 0.0)
                        nc.vector._custom_dve(
                            ttmax,
                            out=junk[:, 0 : w // 2],
                            in0=dbuf[:, 0 : w // 2],
                            in1=dbuf[:, w // 2 : w],
                            accum_out=rows[:, t : t + 1],
                        )
                    if t > 0:
                        nc.vector.tensor_max(acc[:, lo:Q], acc[:, lo:Q], dbuf)

                def finalize(h, pt_tag):
                    acc = acc0 if h == 0 else acc1
                    cols = cols0 if h == 0 else cols1
                    rows = rows0 if h == 0 else rows1
                    # transpose the colmax surface into fp16 PSUM (borrowing
                    # an idle pk buffer), reduce over original partitions
                    pt = psK.tile([128, Q], f16, tag=pt_tag, name=f"pt{h}")
                    for t in range(NT):
                        nc.tensor.transpose(
                            pt[:, 128 * t : 128 * (t + 1)],
                            acc[:, 128 * t : 128 * (t + 1)],
                            ident16,
                        )
                        if t % 4 == 3:
                            c0 = t - 3
                            nc.vector.tensor_reduce(
                                out=cols[:, c0 : t + 1],
                                in_=pt[:, 128 * c0 : 128 * (t + 1)].rearrange(
                                    "p (t c) -> p t c", c=128),
                                axis=AX.X, op=ALU.max,
                            )
                    nc.vector.tensor_scalar_max(cols, cols, 0.0)
                    nc.vector.tensor_scalar_max(rows, rows, 0.0)

                # h0's pt borrows pk1 (its last user, strip t6, frees it at
                # the drain): only h1's second strip (the next pk1 user)
                # waits on the h0 reduces, and the pipeline absorbs that.
                for k, t in enumerate(ORDER):
                    strip(0, t, k)
                finalize(0, "pk0")
                # h0's softmax inputs while h1 streams (exp also pulls the
                # ACT table load off the critical tail)
                nc.vector.tensor_add(alpha_seg[:, 0:10], rows0[:, 0:10], cols0[:, 0:10])
                nc.vector.tensor_add(alpha_seg[:, 20:27], rows0[:, 9:16], cols0[:, 9:16])
                nc.vector.tensor_add(alpha_seg[:, 9:10], alpha_seg[:, 9:10], bm[:, 0:1])
                nc.vector.tensor_add(alpha_seg[:, 20:21], alpha_seg[:, 20:21], bm[:, 1:2])
                nc.scalar.activation(
                    out=w34[:, 0:10], in_=alpha_seg[:, 0:10], func=ACTF.Exp,
                    scale=1.0, accum_out=s_pm[:, 0:1],
                )
                nc.scalar.activation(
                    out=w34[:, 20:27], in_=alpha_seg[:, 20:27], func=ACTF.Exp,
                    scale=1.0, accum_out=s_pm[:, 2:3],
                )
                nc.vector.tensor_copy(w2[:, 0:10, 0], w34[:, 0:10])
                nc.vector.tensor_copy(w2[:, 10:17, 0], w34[:, 20:27])
                for k, t in enumerate(ORDER):
                    strip(1, t, k)
                finalize(1, "pk0")  # pk0 last used by t14

            # late inputs (tail only)
            m_sb = sb.tile([128, NT, C], f32, tag="m_sb")
            nc.sync.dma_start(
                out=m_sb, in_=m_d[:, :].rearrange("(t p) c -> p t c", p=128)
            )
            # ---- softmax tail (h1 share; h0's was hoisted above) ----
            # alpha, segment-aligned cols: [h0s1 0:10 | h1s1 10:20 |
            # h0s2 20:27 | h1s2 27:34]; boundary row 1200 = tile 9 part 48.
            # h0's segments (and their exp) are emitted right after h0's
            # reduces so only h1's share sits on the critical tail.
            r_sb = sb.tile([64, 4], f32, tag="r_sb")

            with tc.tile_pool(name="psF", bufs=1, space="PSUM") as psF:
                nc.vector.tensor_add(alpha_seg[:, 10:20], rows1[:, 0:10], cols1[:, 0:10])
                nc.vector.tensor_add(alpha_seg[:, 27:34], rows1[:, 9:16], cols1[:, 9:16])
                # kill the out-of-segment halves of boundary tile 9 by adding
                # -3e38 (host mask; DVE ops cannot start at partition 48)
                nc.vector.tensor_add(alpha_seg[:, 19:20], alpha_seg[:, 19:20], bm[:, 0:1])
                nc.vector.tensor_add(alpha_seg[:, 27:28], alpha_seg[:, 27:28], bm[:, 1:2])

                # alpha >= 0 and bounded far below fp32 exp overflow for
                # randn-scale inputs, so softmax needs no max-subtraction:
                # exp(alpha)/sum is identical
                for k, (a, b) in ((1, (10, 20)), (3, (27, 34))):
                    nc.scalar.activation(
                        out=w34[:, a:b], in_=alpha_seg[:, a:b], func=ACTF.Exp,
                        scale=1.0,
                        accum_out=s_pm[:, k : k + 1],
                    )
                # interleave weights so each M-tile's (h0, h1) pair is one
                # contiguous [128, 2] matmul rhs
                nc.vector.tensor_copy(w2[:, 0:10, 1], w34[:, 10:20])
                nc.vector.tensor_copy(w2[:, 10:17, 1], w34[:, 27:34])

                r1p = psF.tile([64, 2], f32, tag="r1p")
                r2p = psF.tile([64, 2], f32, tag="r2p")
                for t in range(10):
                    nc.tensor.matmul(
                        r1p[:, :], m_sb[:, t, :], w2[:, t, :],
                        start=(t == 0), stop=(t == 9),
                    )
                for t in range(7):
                    nc.tensor.matmul(
                        r2p[:, :], m_sb[:, 9 + t, :], w2[:, 10 + t, :],
                        start=(t == 0), stop=(t == 6),
                    )
                nc.vector.tensor_copy(r_sb[:, 0:2], r1p[:, :])
                nc.vector.tensor_copy(r_sb[:, 2:4], r2p[:, :])
                nc.sync.dma_start(out=out_d[:, :], in_=r_sb)
                nc.scalar.dma_start(out=spm_d[:, :], in_=s_pm)
                if _CACHE.get("debug"):
                    acc0_32 = sb.tile([128, Q], f32, tag="acc0_32")
                    acc1_32 = sb.tile([128, Q], f32, tag="acc1_32")
                    nc.vector.tensor_copy(acc0_32, acc0)
                    nc.vector.tensor_copy(acc1_32, acc1)
                    nc.sync.dma_start(out=dbg_rows[:, 0, :], in_=rows0)
                    nc.sync.dma_start(out=dbg_rows[:, 1, :], in_=rows1)
                    nc.sync.dma_start(out=dbg_cols[:, 0, :], in_=cols0)
                    nc.sync.dma_start(out=dbg_cols[:, 1, :], in_=cols1)
                    nc.sync.dma_start(out=dbg_acc[:, 0, :], in_=acc0_32)
                    nc.sync.dma_start(out=dbg_acc[:, 1, :], in_=acc1_32)

    return nc


def _get_nc():
    if "nc" not in _CACHE:
        _CACHE["nc"] = _patch_bass_json(_build_nc())
    return _CACHE["nc"]


def _host_inputs(x1, x2, U):
    x1 = np.asarray(x1, dtype=np.float32)
    x2 = np.asarray(x2, dtype=np.float32)
    U = np.asarray(U, dtype=np.float32)
    us = (U * (C ** -0.5)).astype(np.float32)

    p = np.arange(128)
    bm = np.zeros((128, 2), np.float32)
    bm[:, 0] = np.where(p >= L1 - 9 * 128, -3.0e38, 0.0)  # seg1 tile9: kill p>=48
    bm[:, 1] = np.where(p < L1 - 9 * 128, -3.0e38, 0.0)   # seg2 tile9: kill p<48

    in_maps = []
    for b in range(B):
        x2p = np.zeros((L2, C), np.float32)
        x2p[:, :D2] = x2[b]
        M = np.concatenate([x1[b], x2p], axis=0)  # [2048, 64]
        at = np.empty((C, 2, Q), np.float32)
        at[:, 0, :] = (M @ us[0]).T
        at[:, 1, :] = (M @ us[1]).T
        in_maps.append(
            {
                "mt_in": np.ascontiguousarray(M.T),
                "m_in": np.ascontiguousarray(M),
                "at_in": at,
                "bmask_in": bm,
            }
        )
    return in_maps


def run_cores(x1, x2, U, **kw):
    """Run on 8 cores; returns BassKernelResults."""
    from concourse.bass_utils import run_bass_kernel_spmd

    nc = _get_nc()
    in_maps = _host_inputs(x1, x2, U)
    return run_bass_kernel_spmd(nc, in_maps, core_ids=list(range(B)), **kw)


def kernel(x1, x2, U):
    res = run_cores(x1, x2, U)
    r1 = np.zeros((B, H, C), np.float32)
    r2 = np.zeros((B, H, C), np.float32)
    for b in range(B):
        rsb = np.asarray(res.results[b]["out"], np.float32)    # [64, 4]
        spm = np.asarray(res.results[b]["spm"], np.float32)    # [128, 4]
        ssum = spm.sum(axis=0)
        r1[b] = (rsb[:, 0:2] / ssum[0:2]).T
        r2[b] = (rsb[:, 2:4] / ssum[2:4]).T
    return r1, r2


# revision 13
# speedup vs baseline: 1.0423x; 1.0423x over previous
"""Trainium2 Bass kernel for nn_DINA_25503515804209 (sparse_attention).

Math (per batch b, head h):
  M = concat(x1, pad(x2)) in R^{2048 x 64}
  K = (1/8) * M U_h M^T          (2048 x 2048)
  rows_i = max(0, max_{p in allowed(i)} K[i,p])
  cols_p = max(0, max_{i in allowed(p)} K[i,p])
    (leading 848x848 block masked; the reference's mask fill value
     min(relu(K_head0)) is 0 for any real input since relu >= 0 and some
     entry is always <= 0 -- the max(0, .) floor implements it exactly)
  alpha = rows + cols; w1 = softmax(alpha[:1200]); w2 = softmax(alpha[1200:])
  r1 = w1 @ M[:1200]; r2 = w2 @ M[1200:]

Sharding: data-parallel over batch B=8 across the 8 NeuronCores.

Engine split per core (DVE is the bottleneck; everything else feeds it):
  PE   strip matmuls (f32r, both heads) + finalize transposes + tail matmuls
  ACT  PSUM->SBUF fp16 drains of every strip + softmax exp
  DVE  rowmax (2-port ttmax custom op), colmax accumulate (fp16 2x
       tensor_max), finalize reduces, small tail ops
  Pool acc[0:848] seeds and the boundary-strip mask zeroing (memsets)
"""

import json

import numpy as np

B, L1, D1, L2, D2, H, C = 8, 1200, 64, 848, 48, 2, 64
Q = L1 + L2            # 2048
NT = Q // 128          # 16 row tiles
MASKED = L2            # leading 848x848 block is masked
NRESTR = 6             # row tiles 0..5 lie fully inside the masked rows
BND = 6                # row tile 6 straddles the mask boundary (row 848)
BP = MASKED - BND * 128  # = 80: partitions 0:80 of tile 6 are masked rows

_CACHE = {}


# --------------------------------------------------------------------------
# BIR post-processing: this walrus build encodes at most one semaphore wait
# per instruction; Tile emits multi-wait sync_infos.  Hoist excess waits
# into preceding same-engine EventSemaphore instructions (what wait_ge
# emits) -- engine sequencers execute in order, so semantics are identical.
# Also run codegen_inst_isa_subclasses, which populates .instr bytes for
# InstISA subclasses (custom DVE ops); raw Bass does not run that pass and
# walrus fails with "ISA wrong length" on empty instr arrays.
# --------------------------------------------------------------------------
def _split_waits_json(j):
    for fn in j.get("functions", []):
        for blk in fn.get("blocks", []):
            insts = blk.get("instructions")
            if not insts:
                continue
            out = []
            for ins in insts:
                si = ins.get("sync_info")
                waits = (si or {}).get("on_wait") or []
                if len(waits) > 1:
                    for k, wt in enumerate(waits[:-1]):
                        out.append(
                            {
                                "debug": ins.get("debug"),
                                "engine": ins["engine"],
                                "ins": [],
                                "name": f"{ins['name']}_hw{k}",
                                "opcode": "EventSemaphore",
                                "outs": [],
                                "sync_info": {"on_update": [], "on_wait": [wt]},
                            }
                        )
                    si["on_wait"] = waits[-1:]
                ups = (si or {}).get("on_update") or []
                if len(ups) > 1:
                    raise RuntimeError(
                        f"instruction {ins['name']} has {len(ups)} updates"
                    )
                out.append(ins)
            blk["instructions"] = out


def _patch_bass_json(nc):
    import concourse.mybir as mybir

    orig = nc.to_json_bytes
    done = []

    def to_json_bytes_patched():
        if not done:
            mybir.codegen_inst_isa_subclasses(nc)
            done.append(True)
        j = json.loads(orig())
        _split_waits_json(j)
        return json.dumps(j).encode()

    nc.to_json_bytes = to_json_bytes_patched
    return nc


def _ttmax_reduce_op():
    """Fused  out = max(in0, in1);  accum_out = rowmax(out)  custom DVE op.

    Consumes two fp16 streams per cycle (both DVE read ports), so one
    instruction replaces the whole pairwise row-max tree of a strip.
    Registered at runtime through dve_ops' documented extension point
    (the uop program ships in the per-NEFF DVE table)."""
    import numpy as np
    import concourse.dve_ops as dve_ops
    from concourse.dve_spec import Spec, Src0, Src1, maxx, lower
    from concourse.dve_table_gen import dve_ver_for
    from concourse.dve_uop import DveOpSpec

    NAME = "TT_MAX_ROWMAX_ANT"
    if NAME in dve_ops._SUB_OPCODE_FOR_NAME:
        return next(op for op in dve_ops.OPS if op.name == NAME)

    def _ref(in0, in1, c0, c1, c2):
        body = np.maximum(in0.astype(np.float32), in1.astype(np.float32))
        return body, body.reshape(body.shape[0], -1).max(axis=-1, keepdims=True)

    spec = Spec(body=maxx(Src0, Src1), accum=maxx, reference=_ref)
    row = dve_ops._CUSTOM_DVE_ROW_BASE + len(dve_ops.OPS)
    ver = dve_ver_for("TRN2")
    sha = DveOpSpec(
        name=NAME, opcode=row, uops=lower(spec, ver=ver), rd1_en=True
    ).sha(ver)
    op = dve_ops.DveOp(NAME, spec, subdim=False, uops_sha={ver: sha})
    dve_ops.OPS.append(op)
    dve_ops._SUB_OPCODE_FOR_NAME[NAME] = row
    dve_ops.CUSTOM_DVE_SPECS[NAME] = spec
    return op


def _build_nc():
    import concourse.bass as bass
    import concourse.mybir as mybir
    import concourse.tile as tile
    from concourse import bass_isa, library_config
    from concourse.dve_ops import TENSOR_MASK_REDUCE
    from concourse.masks import make_identity

    ttmax = _ttmax_reduce_op()

    f32 = mybir.dt.float32
    f32r = mybir.dt.float32r
    f16 = mybir.dt.float16
    AX = mybir.AxisListType
    ALU = mybir.AluOpType
    ACTF = mybir.ActivationFunctionType

    nc = bass.Bass(trn_type="TRN2")

    mt_d = nc.dram_tensor("mt_in", [C, Q], f32, kind="ExternalInput")
    m_d = nc.dram_tensor("m_in", [Q, C], f32, kind="ExternalInput")
    at_d = nc.dram_tensor("at_in", [C, 2, Q], f32, kind="ExternalInput")
    bm_d = nc.dram_tensor("bmask_in", [128, 2], f32, kind="ExternalInput")
    out_d = nc.dram_tensor("out", [C, 4], f32, kind="ExternalOutput")
    spm_d = nc.dram_tensor("spm", [128, 4], f32, kind="ExternalOutput")
    if _CACHE.get("debug"):
        dbg_rows = nc.dram_tensor("dbg_rows", [128, 2, NT], f32, kind="ExternalOutput")
        dbg_cols = nc.dram_tensor("dbg_cols", [128, 2, NT], f32, kind="ExternalOutput")
        dbg_acc = nc.dram_tensor("dbg_acc", [128, 2, Q], f32, kind="ExternalOutput")

    with tile.TileContext(nc) as tc:
        with (
            tc.tile_pool(name="sb", bufs=1) as sb,
            tc.tile_pool(name="dscr", bufs=4) as dscr,
        ):
            # ---- load inputs (f32r tiles loaded directly; PE rounds).
            # A^T = (M U_h)^T is precomputed on the host (33 MFLOP) so the
            # strip matmuls start as soon as the first DMA chunks land.
            # Order: what strip t0 (restricted, cols 848:) needs comes first.
            nc.gpsimd.load_library(library_config.attn)
            mtr = sb.tile([C, Q], f32r, tag="mtr")
            atr = sb.tile([C, 2, Q], f32r, tag="atr")
            # strict priority order, single queue: the first strip needs
            # atr[:, :, 0:256] and mtr banks 1-3; everything else after.
            nc.sync.dma_start(
                out=atr[:, :, 0:256], in_=at_d[:, :, 0:256].bitcast(f32r)
            )
            nc.sync.dma_start(out=mtr[:, 512:Q], in_=mt_d[:, 512:Q].bitcast(f32r))
            nc.sync.dma_start(
                out=atr[:, :, 256:1024], in_=at_d[:, :, 256:1024].bitcast(f32r)
            )
            nc.sync.dma_start(
                out=atr[:, :, 1024:Q], in_=at_d[:, :, 1024:Q].bitcast(f32r)
            )
            nc.sync.dma_start(out=mtr[:, 0:512], in_=mt_d[:, 0:512].bitcast(f32r))

            e1200 = sb.tile([128, 1], f32, tag="e1200")
            nc.vector.memset(e1200, float(Q - MASKED))
            bm = sb.tile([128, 2], f32, tag="bm")
            nc.sync.dma_start(out=bm, in_=bm_d[:, :])
            ident16 = sb.tile([128, 128], f16, tag="ident16")
            make_identity(nc, ident16)
            ident32 = sb.tile([128, 128], f32, tag="ident32")
            make_identity(nc, ident32)

            # ---- per-head strip processing -------------------------------
            # rows: 2-port ttmax over the ACT-drained fp16 copy (except the
            # first two h0 strips, which use the 1-port TENSOR_MASK_REDUCE
            # directly on PSUM so the DVE has work before the ACT drain
            # pipeline fills).  cols: fp16 2x tensor_max accumulation into a
            # per-head surface, seeded by strip t0's drain (cols 848:) and a
            # Pool memset (cols 0:848).  Boundary strip t6 drains fully and
            # Pool zeroes its masked block; it is processed LAST in the head
            # so the extra Pool hop never blocks the DVE queue.
            rows0 = sb.tile([128, NT], f32, tag="rows0")
            rows1 = sb.tile([128, NT], f32, tag="rows1")
            cols0 = sb.tile([128, NT], f32, tag="cols0")
            cols1 = sb.tile([128, NT], f32, tag="cols1")
            acc0 = sb.tile([128, Q], f16, tag="acc0")
            acc1 = sb.tile([128, Q], f16, tag="acc1")
            junk = sb.tile([128, Q // 2], f16, tag="junk")
            colv0 = sb.tile([128, Q], f32, tag="colv0")
            cols_tmp0 = sb.tile([NT, 128], f32, tag="cols_tmp0")
            nc.gpsimd.memset(acc0[:, 0:MASKED], 0.0)
            nc.gpsimd.memset(acc1[:, 0:MASKED], 0.0)

            ORDER = [0, 1, 2, 3, 4, 5, 7, 8, 9, 10, BND, 11, 12, 13, 14, 15]

            alpha_seg = sb.tile([128, 34], f32, tag="alpha_seg")
            s_pm = sb.tile([128, 4], f32, tag="s_pm")
            w2 = sb.tile([128, 17, 2], f32, tag="w2")

            with tc.tile_pool(name="psK", bufs=1, space="PSUM") as psK:
                def strip(h, t, k):
                    """k = position in processing order (for psum parity)."""
                    ramp = 4 if h == 0 else 2
                    acc = acc0 if h == 0 else acc1
                    rows = rows0 if h == 0 else rows1
                    isl = slice(128 * t, 128 * (t + 1))
                    lo = MASKED if t < NRESTR else 0
                    mmlo = 512 if t < NRESTR else 0
                    w = Q - lo
                    pkf = psK.tile([128, Q], f32, tag=f"pk{k % 2}",
                                   name=f"pk_{h}_{t}")
                    pk = pkf[:, lo:Q]
                    for j in range(mmlo // 512, 4):
                        nc.tensor.matmul(
                            pkf[:, 512 * j : 512 * (j + 1)],
                            atr[:, h, isl],
                            mtr[:, 512 * j : 512 * (j + 1)],
                            start=True, stop=True,
                        )
                    if k < ramp:
                        # ramp: masked-reduce straight from PSUM (drain +
                        # rowmax in one DVE op, no ACT dependency)
                        if t == 0:
                            dbuf = acc[:, lo:Q]
                        else:
                            dbuf = dscr.tile([128, Q], f16, tag="d",
                                             name=f"d_{h}_{t}")[:, lo:Q]
                        nc.vector._custom_dve(
                            TENSOR_MASK_REDUCE,
                            out=dbuf, in0=pk[:, :], in1=e1200,
                            s0=0.0, s1=0.0, imm2=1.0,
                            accum_out=rows[:, t : t + 1],
                        )
                    else:
                        if t == 0:
                            dbuf = acc[:, lo:Q]
                        else:
                            dbuf = dscr.tile([128, Q], f16, tag="d",
                                             name=f"d_{h}_{t}")[:, lo:Q]
                        nc.scalar.copy(dbuf, pk[:, :])
                        if t == BND:
                            # zero the masked block (rows 768:848 x cols
                            # 0:848); 0 is max-neutral after the relu floor.
                            nc.gpsimd.memset(dbuf[0:64, 0:MASKED], 0.0)
                            nc.gpsimd.memset(dbuf[64:BP, 0:MASKED], 0.0)
                        nc.vector._custom_dve(
                            ttmax,
                            out=junk[:, 0 : w // 2],
                            in0=dbuf[:, 0 : w // 2],
                            in1=dbuf[:, w // 2 : w],
                            accum_out=rows[:, t : t + 1],
                        )
                    if t > 0:
                        nc.vector.tensor_max(acc[:, lo:Q], acc[:, lo:Q], dbuf)

                def finalize(h, pt_tag):
                    acc = acc0 if h == 0 else acc1
                    cols = cols0 if h == 0 else cols1
                    rows = rows0 if h == 0 else rows1
                    # transpose the colmax surface into fp16 PSUM (borrowing
                    # an idle pk buffer), reduce over original partitions
                    pt = psK.tile([128, Q], f16, tag=pt_tag, name=f"pt{h}")
                    for t in range(NT):
                        nc.tensor.transpose(
                            pt[:, 128 * t : 128 * (t + 1)],
                            acc[:, 128 * t : 128 * (t + 1)],
                            ident16,
                        )
                        if t % 4 == 3:
                            c0 = t - 3
                            nc.vector.tensor_reduce(
                                out=cols[:, c0 : t + 1],
                                in_=pt[:, 128 * c0 : 128 * (t + 1)].rearrange(
                                    "p (t c) -> p t c", c=128),
                                axis=AX.X, op=ALU.max,
                            )
                    nc.vector.tensor_scalar_max(cols, cols, 0.0)
                    nc.vector.tensor_scalar_max(rows, rows, 0.0)

                # h0's pt borrows pk1 (its last user, strip t6, frees it at
                # the drain): only h1's second strip (the next pk1 user)
                # waits on the h0 reduces, and the pipeline absorbs that.
                for k, t in enumerate(ORDER):
                    strip(0, t, k)
                # h0 colmax finalize rides the idle Pool engine + DMA while
                # h1 streams; only one tiny PE transpose touches PSUM.
                nc.gpsimd.partition_all_reduce(
                    colv0, acc0, channels=128, reduce_op=bass_isa.ReduceOp.max
                )
                nc.sync.dma_start(out=cols_tmp0, in_=colv0[0:1, :])
                for k, t in enumerate(ORDER):
                    strip(1, t, k)
                    if k == 4:
                        # [16, 128] -> [128, 16] via one PE transpose into a
                        # briefly-borrowed pk1 slot
                        ptiny = psK.tile([128, Q], f32, tag="pk1",
                                         name="ptiny")
                        nc.tensor.transpose(
                            ptiny[0:128, 0:NT], cols_tmp0, ident32[0:NT, 0:NT]
                        )
                        nc.vector.tensor_scalar_max(cols0, ptiny[0:128, 0:NT], 0.0)
                        nc.vector.tensor_scalar_max(rows0, rows0, 0.0)
                        nc.vector.tensor_add(alpha_seg[:, 0:10], rows0[:, 0:10], cols0[:, 0:10])
                        nc.vector.tensor_add(alpha_seg[:, 20:27], rows0[:, 9:16], cols0[:, 9:16])
                        nc.vector.tensor_add(alpha_seg[:, 9:10], alpha_seg[:, 9:10], bm[:, 0:1])
                        nc.vector.tensor_add(alpha_seg[:, 20:21], alpha_seg[:, 20:21], bm[:, 1:2])
                        nc.scalar.activation(
                            out=w2[:, 0:10, 0], in_=alpha_seg[:, 0:10],
                            func=ACTF.Exp, scale=1.0, accum_out=s_pm[:, 0:1],
                        )
                        nc.scalar.activation(
                            out=w2[:, 10:17, 0], in_=alpha_seg[:, 20:27],
                            func=ACTF.Exp, scale=1.0, accum_out=s_pm[:, 2:3],
                        )
                finalize(1, "pk0")  # pk0 last used by t14

            # late inputs (tail only)
            m_sb = sb.tile([128, NT, C], f32, tag="m_sb")
            nc.sync.dma_start(
                out=m_sb, in_=m_d[:, :].rearrange("(t p) c -> p t c", p=128)
            )
            # ---- softmax tail (h1 share; h0's was hoisted above) ----
            # alpha, segment-aligned cols: [h0s1 0:10 | h1s1 10:20 |
            # h0s2 20:27 | h1s2 27:34]; boundary row 1200 = tile 9 part 48.
            # h0's segments (and their exp) are emitted right after h0's
            # reduces so only h1's share sits on the critical tail.
            r_sb = sb.tile([64, 4], f32, tag="r_sb")

            with tc.tile_pool(name="psF", bufs=1, space="PSUM") as psF:
                nc.vector.tensor_add(alpha_seg[:, 10:20], rows1[:, 0:10], cols1[:, 0:10])
                nc.vector.tensor_add(alpha_seg[:, 27:34], rows1[:, 9:16], cols1[:, 9:16])
                # kill the out-of-segment halves of boundary tile 9 by adding
                # -3e38 (host mask; DVE ops cannot start at partition 48)
                nc.vector.tensor_add(alpha_seg[:, 19:20], alpha_seg[:, 19:20], bm[:, 0:1])
                nc.vector.tensor_add(alpha_seg[:, 27:28], alpha_seg[:, 27:28], bm[:, 1:2])

                # alpha >= 0 and bounded far below fp32 exp overflow for
                # randn-scale inputs, so softmax needs no max-subtraction:
                # exp(alpha)/sum is identical
                nc.scalar.activation(
                    out=w2[:, 0:10, 1], in_=alpha_seg[:, 10:20],
                    func=ACTF.Exp, scale=1.0, accum_out=s_pm[:, 1:2],
                )
                nc.scalar.activation(
                    out=w2[:, 10:17, 1], in_=alpha_seg[:, 27:34],
                    func=ACTF.Exp, scale=1.0, accum_out=s_pm[:, 3:4],
                )

                r1p = psF.tile([64, 2], f32, tag="r1p")
                r2p = psF.tile([64, 2], f32, tag="r2p")
                for t in range(10):
                    nc.tensor.matmul(
                        r1p[:, :], m_sb[:, t, :], w2[:, t, :],
                        start=(t == 0), stop=(t == 9),
                    )
                for t in range(7):
                    nc.tensor.matmul(
                        r2p[:, :], m_sb[:, 9 + t, :], w2[:, 10 + t, :],
                        start=(t == 0), stop=(t == 6),
                    )
                nc.vector.tensor_copy(r_sb[:, 0:2], r1p[:, :])
                nc.vector.tensor_copy(r_sb[:, 2:4], r2p[:, :])
                nc.sync.dma_start(out=out_d[:, :], in_=r_sb)
                nc.scalar.dma_start(out=spm_d[:, :], in_=s_pm)
                if _CACHE.get("debug"):
                    acc0_32 = sb.tile([128, Q], f32, tag="acc0_32")
                    acc1_32 = sb.tile([128, Q], f32, tag="acc1_32")
                    nc.vector.tensor_copy(acc0_32, acc0)
                    nc.vector.tensor_copy(acc1_32, acc1)
                    nc.sync.dma_start(out=dbg_rows[:, 0, :], in_=rows0)
                    nc.sync.dma_start(out=dbg_rows[:, 1, :], in_=rows1)
                    nc.sync.dma_start(out=dbg_cols[:, 0, :], in_=cols0)
                    nc.sync.dma_start(out=dbg_cols[:, 1, :], in_=cols1)
                    nc.sync.dma_start(out=dbg_acc[:, 0, :], in_=acc0_32)
                    nc.sync.dma_start(out=dbg_acc[:, 1, :], in_=acc1_32)

    return nc


def _get_nc():
    if "nc" not in _CACHE:
        _CACHE["nc"] = _patch_bass_json(_build_nc())
    return _CACHE["nc"]


def _host_inputs(x1, x2, U):
    x1 = np.asarray(x1, dtype=np.float32)
    x2 = np.asarray(x2, dtype=np.float32)
    U = np.asarray(U, dtype=np.float32)
    us = (U * (C ** -0.5)).astype(np.float32)

    p = np.arange(128)
    bm = np.zeros((128, 2), np.float32)
    bm[:, 0] = np.where(p >= L1 - 9 * 128, -3.0e38, 0.0)  # seg1 tile9: kill p>=48
    bm[:, 1] = np.where(p < L1 - 9 * 128, -3.0e38, 0.0)   # seg2 tile9: kill p<48

    in_maps = []
    for b in range(B):
        x2p = np.zeros((L2, C), np.float32)
        x2p[:, :D2] = x2[b]
        M = np.concatenate([x1[b], x2p], axis=0)  # [2048, 64]
        at = np.empty((C, 2, Q), np.float32)
        at[:, 0, :] = (M @ us[0]).T
        at[:, 1, :] = (M @ us[1]).T
        in_maps.append(
            {
                "mt_in": np.ascontiguousarray(M.T),
                "m_in": np.ascontiguousarray(M),
                "at_in": at,
                "bmask_in": bm,
            }
        )
    return in_maps


def run_cores(x1, x2, U, **kw):
    """Run on 8 cores; returns BassKernelResults."""
    from concourse.bass_utils import run_bass_kernel_spmd

    nc = _get_nc()
    in_maps = _host_inputs(x1, x2, U)
    return run_bass_kernel_spmd(nc, in_maps, core_ids=list(range(B)), **kw)


def kernel(x1, x2, U):
    res = run_cores(x1, x2, U)
    r1 = np.zeros((B, H, C), np.float32)
    r2 = np.zeros((B, H, C), np.float32)
    for b in range(B):
        rsb = np.asarray(res.results[b]["out"], np.float32)    # [64, 4]
        spm = np.asarray(res.results[b]["spm"], np.float32)    # [128, 4]
        ssum = spm.sum(axis=0)
        r1[b] = (rsb[:, 0:2] / ssum[0:2]).T
        r2[b] = (rsb[:, 2:4] / ssum[2:4]).T
    return r1, r2


# revision 14
# speedup vs baseline: 1.0472x; 1.0047x over previous
"""Trainium2 Bass kernel for nn_DINA_25503515804209 (sparse_attention).

Math (per batch b, head h):
  M = concat(x1, pad(x2)) in R^{2048 x 64}
  K = (1/8) * M U_h M^T          (2048 x 2048)
  rows_i = max(0, max_{p in allowed(i)} K[i,p])
  cols_p = max(0, max_{i in allowed(p)} K[i,p])
    (leading 848x848 block masked; the reference's mask fill value
     min(relu(K_head0)) is 0 for any real input since relu >= 0 and some
     entry is always <= 0 -- the max(0, .) floor implements it exactly)
  alpha = rows + cols; w1 = softmax(alpha[:1200]); w2 = softmax(alpha[1200:])
  r1 = w1 @ M[:1200]; r2 = w2 @ M[1200:]

Sharding: data-parallel over batch B=8 across the 8 NeuronCores.

Engine split per core (DVE is the bottleneck; everything else feeds it):
  PE   strip matmuls (f32r, both heads) + finalize transposes + tail matmuls
  ACT  PSUM->SBUF fp16 drains of every strip + softmax exp
  DVE  rowmax (2-port ttmax custom op), colmax accumulate (fp16 2x
       tensor_max), finalize reduces, small tail ops
  Pool acc[0:848] seeds and the boundary-strip mask zeroing (memsets)
"""

import json

import numpy as np

B, L1, D1, L2, D2, H, C = 8, 1200, 64, 848, 48, 2, 64
Q = L1 + L2            # 2048
NT = Q // 128          # 16 row tiles
MASKED = L2            # leading 848x848 block is masked
NRESTR = 6             # row tiles 0..5 lie fully inside the masked rows
BND = 6                # row tile 6 straddles the mask boundary (row 848)
BP = MASKED - BND * 128  # = 80: partitions 0:80 of tile 6 are masked rows

_CACHE = {}


# --------------------------------------------------------------------------
# BIR post-processing: this walrus build encodes at most one semaphore wait
# per instruction; Tile emits multi-wait sync_infos.  Hoist excess waits
# into preceding same-engine EventSemaphore instructions (what wait_ge
# emits) -- engine sequencers execute in order, so semantics are identical.
# Also run codegen_inst_isa_subclasses, which populates .instr bytes for
# InstISA subclasses (custom DVE ops); raw Bass does not run that pass and
# walrus fails with "ISA wrong length" on empty instr arrays.
# --------------------------------------------------------------------------
def _split_waits_json(j):
    for fn in j.get("functions", []):
        for blk in fn.get("blocks", []):
            insts = blk.get("instructions")
            if not insts:
                continue
            out = []
            for ins in insts:
                si = ins.get("sync_info")
                waits = (si or {}).get("on_wait") or []
                if len(waits) > 1:
                    for k, wt in enumerate(waits[:-1]):
                        out.append(
                            {
                                "debug": ins.get("debug"),
                                "engine": ins["engine"],
                                "ins": [],
                                "name": f"{ins['name']}_hw{k}",
                                "opcode": "EventSemaphore",
                                "outs": [],
                                "sync_info": {"on_update": [], "on_wait": [wt]},
                            }
                        )
                    si["on_wait"] = waits[-1:]
                ups = (si or {}).get("on_update") or []
                if len(ups) > 1:
                    raise RuntimeError(
                        f"instruction {ins['name']} has {len(ups)} updates"
                    )
                out.append(ins)
            blk["instructions"] = out


def _patch_bass_json(nc):
    import concourse.mybir as mybir

    orig = nc.to_json_bytes
    done = []

    def to_json_bytes_patched():
        if not done:
            mybir.codegen_inst_isa_subclasses(nc)
            done.append(True)
        j = json.loads(orig())
        _split_waits_json(j)
        return json.dumps(j).encode()

    nc.to_json_bytes = to_json_bytes_patched
    return nc


def _ttmax_reduce_op():
    """Fused  out = max(in0, in1);  accum_out = rowmax(out)  custom DVE op.

    Consumes two fp16 streams per cycle (both DVE read ports), so one
    instruction replaces the whole pairwise row-max tree of a strip.
    Registered at runtime through dve_ops' documented extension point
    (the uop program ships in the per-NEFF DVE table)."""
    import numpy as np
    import concourse.dve_ops as dve_ops
    from concourse.dve_spec import Spec, Src0, Src1, maxx, lower
    from concourse.dve_table_gen import dve_ver_for
    from concourse.dve_uop import DveOpSpec

    NAME = "TT_MAX_ROWMAX_ANT"
    if NAME in dve_ops._SUB_OPCODE_FOR_NAME:
        return next(op for op in dve_ops.OPS if op.name == NAME)

    def _ref(in0, in1, c0, c1, c2):
        body = np.maximum(in0.astype(np.float32), in1.astype(np.float32))
        return body, body.reshape(body.shape[0], -1).max(axis=-1, keepdims=True)

    spec = Spec(body=maxx(Src0, Src1), accum=maxx, reference=_ref)
    row = dve_ops._CUSTOM_DVE_ROW_BASE + len(dve_ops.OPS)
    ver = dve_ver_for("TRN2")
    sha = DveOpSpec(
        name=NAME, opcode=row, uops=lower(spec, ver=ver), rd1_en=True
    ).sha(ver)
    op = dve_ops.DveOp(NAME, spec, subdim=False, uops_sha={ver: sha})
    dve_ops.OPS.append(op)
    dve_ops._SUB_OPCODE_FOR_NAME[NAME] = row
    dve_ops.CUSTOM_DVE_SPECS[NAME] = spec
    return op


def _build_nc():
    import concourse.bass as bass
    import concourse.mybir as mybir
    import concourse.tile as tile
    from concourse import bass_isa, library_config
    from concourse.dve_ops import TENSOR_MASK_REDUCE
    from concourse.masks import make_identity

    ttmax = _ttmax_reduce_op()

    f32 = mybir.dt.float32
    f32r = mybir.dt.float32r
    f16 = mybir.dt.float16
    AX = mybir.AxisListType
    ALU = mybir.AluOpType
    ACTF = mybir.ActivationFunctionType

    nc = bass.Bass(trn_type="TRN2")

    mt_d = nc.dram_tensor("mt_in", [C, Q], f32, kind="ExternalInput")
    m_d = nc.dram_tensor("m_in", [Q, C], f32, kind="ExternalInput")
    at_d = nc.dram_tensor("at_in", [C, 2, Q], f32, kind="ExternalInput")
    bm_d = nc.dram_tensor("bmask_in", [128, 2], f32, kind="ExternalInput")
    out_d = nc.dram_tensor("out", [C, 4], f32, kind="ExternalOutput")
    spm_d = nc.dram_tensor("spm", [128, 4], f32, kind="ExternalOutput")
    if _CACHE.get("debug"):
        dbg_rows = nc.dram_tensor("dbg_rows", [128, 2, NT], f32, kind="ExternalOutput")
        dbg_cols = nc.dram_tensor("dbg_cols", [128, 2, NT], f32, kind="ExternalOutput")
        dbg_acc = nc.dram_tensor("dbg_acc", [128, 2, Q], f32, kind="ExternalOutput")

    with tile.TileContext(nc) as tc:
        with (
            tc.tile_pool(name="sb", bufs=1) as sb,
            tc.tile_pool(name="dscr", bufs=4) as dscr,
        ):
            # ---- load inputs (f32r tiles loaded directly; PE rounds).
            # A^T = (M U_h)^T is precomputed on the host (33 MFLOP) so the
            # strip matmuls start as soon as the first DMA chunks land.
            # Order: what strip t0 (restricted, cols 848:) needs comes first.
            nc.gpsimd.load_library(library_config.attn)
            mtr = sb.tile([C, Q], f32r, tag="mtr")
            atr = sb.tile([C, 2, Q], f32r, tag="atr")
            # strict priority order, single queue: the first strip needs
            # atr[:, :, 0:256] and mtr banks 1-3; everything else after.
            nc.sync.dma_start(
                out=atr[:, :, 0:256], in_=at_d[:, :, 0:256].bitcast(f32r)
            )
            nc.sync.dma_start(out=mtr[:, 512:Q], in_=mt_d[:, 512:Q].bitcast(f32r))
            nc.sync.dma_start(
                out=atr[:, :, 256:1024], in_=at_d[:, :, 256:1024].bitcast(f32r)
            )
            nc.sync.dma_start(
                out=atr[:, :, 1024:Q], in_=at_d[:, :, 1024:Q].bitcast(f32r)
            )
            nc.sync.dma_start(out=mtr[:, 0:512], in_=mt_d[:, 0:512].bitcast(f32r))

            e1200 = sb.tile([128, 1], f32, tag="e1200")
            nc.vector.memset(e1200, float(Q - MASKED))
            bm = sb.tile([128, 2], f32, tag="bm")
            nc.sync.dma_start(out=bm, in_=bm_d[:, :])
            ident16 = sb.tile([128, 128], f16, tag="ident16")
            make_identity(nc, ident16)
            ident32 = sb.tile([128, 128], f32, tag="ident32")
            make_identity(nc, ident32)

            # ---- per-head strip processing -------------------------------
            # rows: 2-port ttmax over the ACT-drained fp16 copy (except the
            # first two h0 strips, which use the 1-port TENSOR_MASK_REDUCE
            # directly on PSUM so the DVE has work before the ACT drain
            # pipeline fills).  cols: fp16 2x tensor_max accumulation into a
            # per-head surface, seeded by strip t0's drain (cols 848:) and a
            # Pool memset (cols 0:848).  Boundary strip t6 drains fully and
            # Pool zeroes its masked block; it is processed LAST in the head
            # so the extra Pool hop never blocks the DVE queue.
            rows0 = sb.tile([128, NT], f32, tag="rows0")
            rows1 = sb.tile([128, NT], f32, tag="rows1")
            cols0 = sb.tile([128, NT], f32, tag="cols0")
            cols1 = sb.tile([128, NT], f32, tag="cols1")
            acc0 = sb.tile([128, Q], f16, tag="acc0")
            acc1 = sb.tile([128, Q], f16, tag="acc1")
            junk = sb.tile([128, Q // 2], f16, tag="junk")
            colv0 = sb.tile([128, Q], f32, tag="colv0")
            cols_tmp0 = sb.tile([NT, 128], f32, tag="cols_tmp0")
            nc.gpsimd.memset(acc0[:, 0:MASKED], 0.0)
            nc.gpsimd.memset(acc1[:, 0:MASKED], 0.0)

            ORDER = [0, 1, 2, 3, 4, 5, 7, 8, 9, 10, BND, 11, 12, 13, 14, 15]

            alpha_seg = sb.tile([128, 34], f32, tag="alpha_seg")
            s_pm = sb.tile([128, 4], f32, tag="s_pm")
            w2 = sb.tile([128, 17, 2], f32, tag="w2")

            with tc.tile_pool(name="psK", bufs=1, space="PSUM") as psK:
                def strip(h, t, k):
                    """k = position in processing order (for psum parity)."""
                    ramp = 4 if h == 0 else 2
                    acc = acc0 if h == 0 else acc1
                    rows = rows0 if h == 0 else rows1
                    isl = slice(128 * t, 128 * (t + 1))
                    lo = MASKED if t < NRESTR else 0
                    mmlo = 512 if t < NRESTR else 0
                    w = Q - lo
                    pkf = psK.tile([128, Q], f32, tag=f"pk{k % 2}",
                                   name=f"pk_{h}_{t}")
                    pk = pkf[:, lo:Q]
                    for j in range(mmlo // 512, 4):
                        nc.tensor.matmul(
                            pkf[:, 512 * j : 512 * (j + 1)],
                            atr[:, h, isl],
                            mtr[:, 512 * j : 512 * (j + 1)],
                            start=True, stop=True,
                        )
                    if k < ramp:
                        # ramp: masked-reduce straight from PSUM (drain +
                        # rowmax in one DVE op, no ACT dependency)
                        if t == 0:
                            dbuf = acc[:, lo:Q]
                        else:
                            dbuf = dscr.tile([128, Q], f16, tag="d",
                                             name=f"d_{h}_{t}")[:, lo:Q]
                        nc.vector._custom_dve(
                            TENSOR_MASK_REDUCE,
                            out=dbuf, in0=pk[:, :], in1=e1200,
                            s0=0.0, s1=0.0, imm2=1.0,
                            accum_out=rows[:, t : t + 1],
                        )
                    else:
                        if t == 0:
                            dbuf = acc[:, lo:Q]
                        else:
                            dbuf = dscr.tile([128, Q], f16, tag="d",
                                             name=f"d_{h}_{t}")[:, lo:Q]
                        nc.scalar.copy(dbuf, pk[:, :])
                        if t == BND:
                            # zero the masked block (rows 768:848 x cols
                            # 0:848); 0 is max-neutral after the relu floor.
                            nc.gpsimd.memset(dbuf[0:64, 0:MASKED], 0.0)
                            nc.gpsimd.memset(dbuf[64:BP, 0:MASKED], 0.0)
                        nc.vector._custom_dve(
                            ttmax,
                            out=junk[:, 0 : w // 2],
                            in0=dbuf[:, 0 : w // 2],
                            in1=dbuf[:, w // 2 : w],
                            accum_out=rows[:, t : t + 1],
                        )
                    if t > 0:
                        nc.vector.tensor_max(acc[:, lo:Q], acc[:, lo:Q], dbuf)

                def finalize(h, pt_tag):
                    acc = acc0 if h == 0 else acc1
                    cols = cols0 if h == 0 else cols1
                    rows = rows0 if h == 0 else rows1
                    # transpose the colmax surface into fp16 PSUM (borrowing
                    # an idle pk buffer), reduce over original partitions
                    pt = psK.tile([128, Q], f16, tag=pt_tag, name=f"pt{h}")
                    for t in range(NT):
                        nc.tensor.transpose(
                            pt[:, 128 * t : 128 * (t + 1)],
                            acc[:, 128 * t : 128 * (t + 1)],
                            ident16,
                        )
                        if t % 4 == 3:
                            c0 = t - 3
                            nc.vector.tensor_reduce(
                                out=cols[:, c0 : t + 1],
                                in_=pt[:, 128 * c0 : 128 * (t + 1)].rearrange(
                                    "p (t c) -> p t c", c=128),
                                axis=AX.X, op=ALU.max,
                            )
                    nc.vector.tensor_scalar_max(cols, cols, 0.0)
                    nc.vector.tensor_scalar_max(rows, rows, 0.0)

                # h0's pt borrows pk1 (its last user, strip t6, frees it at
                # the drain): only h1's second strip (the next pk1 user)
                # waits on the h0 reduces, and the pipeline absorbs that.
                for k, t in enumerate(ORDER):
                    strip(0, t, k)
                # h0 colmax finalize rides the idle Pool engine + DMA while
                # h1 streams; only one tiny PE transpose touches PSUM.
                nc.gpsimd.partition_all_reduce(
                    colv0, acc0, channels=128, reduce_op=bass_isa.ReduceOp.max
                )
                nc.sync.dma_start(out=cols_tmp0, in_=colv0[0:1, :])
                for k, t in enumerate(ORDER):
                    if k == 6:
                        # [16, 128] -> [128, 16] via one PE transpose into a
                        # briefly-borrowed pk1 slot
                        ptiny = psK.tile([128, Q], f32, tag="pk1",
                                         name="ptiny")
                        nc.tensor.transpose(
                            ptiny[0:128, 0:NT], cols_tmp0, ident32[0:NT, 0:NT]
                        )
                        nc.vector.tensor_scalar_max(cols0, ptiny[0:128, 0:NT], 0.0)
                        nc.vector.tensor_scalar_max(rows0, rows0, 0.0)
                        nc.vector.tensor_add(alpha_seg[:, 0:10], rows0[:, 0:10], cols0[:, 0:10])
                        nc.vector.tensor_add(alpha_seg[:, 20:27], rows0[:, 9:16], cols0[:, 9:16])
                        nc.vector.tensor_add(alpha_seg[:, 9:10], alpha_seg[:, 9:10], bm[:, 0:1])
                        nc.vector.tensor_add(alpha_seg[:, 20:21], alpha_seg[:, 20:21], bm[:, 1:2])
                        nc.scalar.activation(
                            out=w2[:, 0:10, 0], in_=alpha_seg[:, 0:10],
                            func=ACTF.Exp, scale=1.0, accum_out=s_pm[:, 0:1],
                        )
                        nc.scalar.activation(
                            out=w2[:, 10:17, 0], in_=alpha_seg[:, 20:27],
                            func=ACTF.Exp, scale=1.0, accum_out=s_pm[:, 2:3],
                        )
                    strip(1, t, k)
                finalize(1, "pk0")  # pk0 last used by t14

            # late inputs (tail only)
            m_sb = sb.tile([128, NT, C], f32, tag="m_sb")
            nc.sync.dma_start(
                out=m_sb, in_=m_d[:, :].rearrange("(t p) c -> p t c", p=128)
            )
            # ---- softmax tail (h1 share; h0's was hoisted above) ----
            # alpha, segment-aligned cols: [h0s1 0:10 | h1s1 10:20 |
            # h0s2 20:27 | h1s2 27:34]; boundary row 1200 = tile 9 part 48.
            # h0's segments (and their exp) are emitted right after h0's
            # reduces so only h1's share sits on the critical tail.
            r_sb = sb.tile([64, 4], f32, tag="r_sb")

            with tc.tile_pool(name="psF", bufs=1, space="PSUM") as psF:
                nc.vector.tensor_add(alpha_seg[:, 10:20], rows1[:, 0:10], cols1[:, 0:10])
                nc.vector.tensor_add(alpha_seg[:, 27:34], rows1[:, 9:16], cols1[:, 9:16])
                # kill the out-of-segment halves of boundary tile 9 by adding
                # -3e38 (host mask; DVE ops cannot start at partition 48)
                nc.vector.tensor_add(alpha_seg[:, 19:20], alpha_seg[:, 19:20], bm[:, 0:1])
                nc.vector.tensor_add(alpha_seg[:, 27:28], alpha_seg[:, 27:28], bm[:, 1:2])

                # alpha >= 0 and bounded far below fp32 exp overflow for
                # randn-scale inputs, so softmax needs no max-subtraction:
                # exp(alpha)/sum is identical
                nc.scalar.activation(
                    out=w2[:, 0:10, 1], in_=alpha_seg[:, 10:20],
                    func=ACTF.Exp, scale=1.0, accum_out=s_pm[:, 1:2],
                )
                nc.scalar.activation(
                    out=w2[:, 10:17, 1], in_=alpha_seg[:, 27:34],
                    func=ACTF.Exp, scale=1.0, accum_out=s_pm[:, 3:4],
                )

                r1p = psF.tile([64, 2], f32, tag="r1p")
                r2p = psF.tile([64, 2], f32, tag="r2p")
                for t in range(10):
                    nc.tensor.matmul(
                        r1p[:, :], m_sb[:, t, :], w2[:, t, :],
                        start=(t == 0), stop=(t == 9),
                    )
                for t in range(7):
                    nc.tensor.matmul(
                        r2p[:, :], m_sb[:, 9 + t, :], w2[:, 10 + t, :],
                        start=(t == 0), stop=(t == 6),
                    )
                nc.vector.tensor_copy(r_sb[:, 0:2], r1p[:, :])
                nc.vector.tensor_copy(r_sb[:, 2:4], r2p[:, :])
                nc.sync.dma_start(out=out_d[:, :], in_=r_sb)
                nc.scalar.dma_start(out=spm_d[:, :], in_=s_pm)
                if _CACHE.get("debug"):
                    acc0_32 = sb.tile([128, Q], f32, tag="acc0_32")
                    acc1_32 = sb.tile([128, Q], f32, tag="acc1_32")
                    nc.vector.tensor_copy(acc0_32, acc0)
                    nc.vector.tensor_copy(acc1_32, acc1)
                    nc.sync.dma_start(out=dbg_rows[:, 0, :], in_=rows0)
                    nc.sync.dma_start(out=dbg_rows[:, 1, :], in_=rows1)
                    nc.sync.dma_start(out=dbg_cols[:, 0, :], in_=cols0)
                    nc.sync.dma_start(out=dbg_cols[:, 1, :], in_=cols1)
                    nc.sync.dma_start(out=dbg_acc[:, 0, :], in_=acc0_32)
                    nc.sync.dma_start(out=dbg_acc[:, 1, :], in_=acc1_32)

    return nc


def _get_nc():
    if "nc" not in _CACHE:
        _CACHE["nc"] = _patch_bass_json(_build_nc())
    return _CACHE["nc"]


def _host_inputs(x1, x2, U):
    x1 = np.asarray(x1, dtype=np.float32)
    x2 = np.asarray(x2, dtype=np.float32)
    U = np.asarray(U, dtype=np.float32)
    us = (U * (C ** -0.5)).astype(np.float32)

    p = np.arange(128)
    bm = np.zeros((128, 2), np.float32)
    bm[:, 0] = np.where(p >= L1 - 9 * 128, -3.0e38, 0.0)  # seg1 tile9: kill p>=48
    bm[:, 1] = np.where(p < L1 - 9 * 128, -3.0e38, 0.0)   # seg2 tile9: kill p<48

    in_maps = []
    for b in range(B):
        x2p = np.zeros((L2, C), np.float32)
        x2p[:, :D2] = x2[b]
        M = np.concatenate([x1[b], x2p], axis=0)  # [2048, 64]
        at = np.empty((C, 2, Q), np.float32)
        at[:, 0, :] = (M @ us[0]).T
        at[:, 1, :] = (M @ us[1]).T
        in_maps.append(
            {
                "mt_in": np.ascontiguousarray(M.T),
                "m_in": np.ascontiguousarray(M),
                "at_in": at,
                "bmask_in": bm,
            }
        )
    return in_maps


def run_cores(x1, x2, U, **kw):
    """Run on 8 cores; returns BassKernelResults."""
    from concourse.bass_utils import run_bass_kernel_spmd

    nc = _get_nc()
    in_maps = _host_inputs(x1, x2, U)
    return run_bass_kernel_spmd(nc, in_maps, core_ids=list(range(B)), **kw)


def kernel(x1, x2, U):
    res = run_cores(x1, x2, U)
    r1 = np.zeros((B, H, C), np.float32)
    r2 = np.zeros((B, H, C), np.float32)
    for b in range(B):
        rsb = np.asarray(res.results[b]["out"], np.float32)    # [64, 4]
        spm = np.asarray(res.results[b]["spm"], np.float32)    # [128, 4]
        ssum = spm.sum(axis=0)
        r1[b] = (rsb[:, 0:2] / ssum[0:2]).T
        r2[b] = (rsb[:, 2:4] / ssum[2:4]).T
    return r1, r2
